# revision 9
# baseline (speedup 1.0000x reference)
"""ANI-style species-routed MoE MLP on 8 Trainium2 NeuronCores.

Strategy:
- Host routing: sort atoms by species (stable argsort), shard each species'
  atoms evenly across 8 cores.  Each core runs ONE expert per atom instead
  of all 7 (the reference's dense masking wastes 7x compute).
- Transposed dataflow: activations live as [feature, atom] so the 3-layer
  MLP chains matmuls without transposes (lhsT = weights stay stationary).
- bf16 matmul inputs, fp32 PSUM accumulation.
- CELU(z, a=0.1) with all scale factors folded into the weights host-side:
  internal activations are G = 10*celu(z) - beta (shifted); per layer
      p    = G_prev @ W            (PSUM, fp32)
      e    = exp(p + beta)         (ScalarE, bias folded)
      u    = min(e,1) + (-1-beta)  (VectorE tensor_scalar, dual-op)
      G    = max(p, u)             (VectorE tensor_tensor)
  where beta absorbs the layer bias AND the shift of the previous layer.
- DMA layout: per-core aev is stored group-contiguous ([126, 8*A] blocks)
  so each group loads with 126 x 16KB descriptors instead of ~1000 x 2KB.
"""

import math

import numpy as np
import ml_dtypes

import concourse.bass as bass
import concourse.mybir as mybir
from concourse import tile
from concourse.bass_utils import run_bass_kernel_spmd

AF = mybir.ActivationFunctionType
ALU = mybir.AluOpType
BF16 = mybir.dt.bfloat16
F32 = mybir.dt.float32
BF16_NP = ml_dtypes.bfloat16

N_CORES = 8
AEV = 1008
KC = 126          # L1 contraction chunk rows (8 * 126 = 1008)
NK1 = 8
OUT_DIM = 2
GROUP_ATOMS = 512

WIDTHS = [(256, 192, 160), (224, 192, 160), (192, 160, 128), (192, 160, 128),
          (160, 128, 96), (160, 128, 96), (160, 128, 96)]
NE = len(WIDTHS)

NM1 = 2                       # w1 padded to 256 for every expert
W1P = NM1 * 128
NM2 = [math.ceil(w2 / 128) for (_, w2, _) in WIDTHS]   # [2,2,2,2,1,1,1]
NM3 = [math.ceil(w3 / 128) for (_, _, w3) in WIDTHS]   # [2,2,1,1,1,1,1]
W2P = [m * 128 for m in NM2]
W3P = [m * 128 for m in NM3]


def _bias_cols():
    cols = {}
    c = 0
    for e in range(NE):
        cols[(e, "b1")] = c; c += NM1
        cols[(e, "n1")] = c; c += NM1
        cols[(e, "b2")] = c; c += NM2[e]
        cols[(e, "n2")] = c; c += NM2[e]
        cols[(e, "b3")] = c; c += NM3[e]
        cols[(e, "n3")] = c; c += NM3[e]
        cols[(e, "bh")] = c; c += 1
    return cols, c


def _w128_cols():
    """Column offsets inside the packed [128, C] weight tensor holding
    W2/W3/Wh blocks for every expert (K-chunk-major per tensor)."""
    cols = {}
    c = 0
    for e in range(NE):
        cols[(e, 2)] = c; c += NM1 * W2P[e]
        cols[(e, 3)] = c; c += NM2[e] * W3P[e]
        cols[(e, 4)] = c; c += NM3[e] * OUT_DIM
    return cols, c


def _legalize_waits(nc, limit=1):
    """This container's walrus accepts at most one sync-wait per
    instruction; split extras into standalone NoOps on the same engine."""
    for fn in nc.m.functions:
        for blk in fn.blocks:
            out, changed = [], False
            for inst in blk.instructions:
                si = inst.sync_info
                waits = list(si.on_wait) if si is not None and si.on_wait else []
                if len(waits) > limit:
                    changed = True
                    for k, w in enumerate(waits[:-limit]):
                        nop = mybir.InstNoOp(name=f"{inst.name}_lw{k}", ins=[], outs=[])
                        nop.engine = inst.engine
                        nop.sync_info = mybir.SyncInfo(on_wait=[w], on_update=[])
                        out.append(nop)
                    upd = list(si.on_update) if si.on_update else []
                    inst.sync_info = mybir.SyncInfo(on_wait=waits[-limit:], on_update=upd)
                out.append(inst)
            if changed:
                blk.instructions = out


def _subs(A):
    return [(o, min(512, A - o)) for o in range(0, A, 512)]


def _build(groups, per_core_n):
    bcols, nbc = _bias_cols()
    wcols, nwc = _w128_cols()
    nc = bass.Bass(trn_type="TRN2")

    aev_d = nc.dram_tensor("aev", (KC, NK1 * per_core_n), BF16, kind="ExternalInput")
    w126_d = nc.dram_tensor("w126", (KC, NE * NK1 * W1P), BF16, kind="ExternalInput")
    w128_d = nc.dram_tensor("w128", (128, nwc), BF16, kind="ExternalInput")
    bias_d = nc.dram_tensor("bias", (128, nbc), F32, kind="ExternalInput")
    out_d = nc.dram_tensor("out", (OUT_DIM, per_core_n), F32, kind="ExternalOutput")

    with tile.TileContext(nc) as tc:
        with (
            tc.tile_pool(name="wpool", bufs=1) as wpool,
            tc.tile_pool(name="xpool", bufs=4) as xpool,
            tc.tile_pool(name="gpool", bufs=3) as gpool,
            tc.tile_pool(name="tpool", bufs=4) as tpool,
            tc.tile_pool(name="hopool", bufs=3) as hopool,
            tc.tile_pool(name="zpool", bufs=6, space="PSUM") as zpool,
            tc.tile_pool(name="hpool", bufs=2, space="PSUM") as hpool,
        ):
            w126_sb = wpool.tile([KC, NE * NK1 * W1P], BF16, tag="w126")
            w128_sb = wpool.tile([128, nwc], BF16, tag="w128")
            bias_sb = wpool.tile([128, nbc], F32, tag="bias")

            def dma_weights(e):
                c0, c1 = e * NK1 * W1P, (e + 1) * NK1 * W1P
                nc.sync.dma_start(w126_sb[:, c0:c1], w126_d[:, c0:c1])
                d0 = wcols[(e, 2)]
                d1 = wcols[(e, 4)] + NM3[e] * OUT_DIM
                nc.sync.dma_start(w128_sb[:, d0:d1], w128_d[:, d0:d1])

            def w1_ap(e, k, m):
                c = e * NK1 * W1P + k * W1P + m * 128
                return w126_sb[:, c:c + 128]

            def wl_ap(e, l, k, m, wp, mw=128):
                c = wcols[(e, l)] + k * wp + m * 128
                return w128_sb[:, c:c + mw]

            def bcol(e, key, m):
                c = bcols[(e, key)] + m
                return bias_sb[:, c:c + 1]

            def celu(z, g_out, e_, key, m, A):
                # e = exp(z + beta); u = min(e,1) + (-1-beta); G = max(z, u)
                e_t = tpool.tile([128, A], BF16, tag="e")
                nc.scalar.activation(e_t[:], z[:], AF.Exp,
                                     bias=bcol(e_, "b" + key, m), scale=1.0)
                u_t = tpool.tile([128, A], BF16, tag="u")
                nc.gpsimd.tensor_scalar(u_t[:], e_t[:], 1.0,
                                        bcol(e_, "n" + key, m), ALU.min, ALU.add)
                nc.vector.tensor_tensor(g_out[:], z[:], u_t[:], ALU.max)

            # Software pipeline: PE stream interleaves stages of different
            # groups so each group's celu chain (EXP->MIN,ADD->MAX) is
            # hidden behind another group's matmuls.
            st = {}   # group idx -> dict(x, g1, g2, g3)

            def prefetch(i):
                (e, off, A) = groups[i]
                x = xpool.tile([KC, NK1, A], BF16, tag="x")
                nc.sync.dma_start(x[:], aev_d[:, NK1 * off:NK1 * (off + A)]
                                  .rearrange("p (k a) -> p k a", k=NK1))
                st[i] = {"x": x}

            def emit_l1(i):
                (e, off, A) = groups[i]
                g1 = gpool.tile([128, NM1, A], BF16, tag="g1")
                for m in range(NM1):
                    z = zpool.tile([128, A], F32, tag="z")
                    for k in range(NK1):
                        for (ao, aw) in _subs(A):
                            nc.tensor.matmul(
                                z[:, ao:ao + aw], w1_ap(e, k, m),
                                st[i]["x"][:, k, ao:ao + aw],
                                start=(k == 0), stop=(k == NK1 - 1))
                    celu(z, g1[:, m, :], e, "1", m, A)
                st[i]["g1"] = g1

            def emit_l2(i):
                (e, off, A) = groups[i]
                g2 = gpool.tile([128, NM2[e], A], BF16, tag="g2")
                for m in range(NM2[e]):
                    z = zpool.tile([128, A], F32, tag="z")
                    for k in range(NM1):
                        for (ao, aw) in _subs(A):
                            nc.tensor.matmul(
                                z[:, ao:ao + aw], wl_ap(e, 2, k, m, W2P[e]),
                                st[i]["g1"][:, k, ao:ao + aw],
                                start=(k == 0), stop=(k == NM1 - 1))
                    celu(z, g2[:, m, :], e, "2", m, A)
                st[i]["g2"] = g2

            def emit_l3(i):
                (e, off, A) = groups[i]
                g3 = gpool.tile([128, NM3[e], A], BF16, tag="g3")
                for m in range(NM3[e]):
                    z = zpool.tile([128, A], F32, tag="z")
                    for k in range(NM2[e]):
                        for (ao, aw) in _subs(A):
                            nc.tensor.matmul(
                                z[:, ao:ao + aw], wl_ap(e, 3, k, m, W3P[e]),
                                st[i]["g2"][:, k, ao:ao + aw],
                                start=(k == 0), stop=(k == NM2[e] - 1))
                    celu(z, g3[:, m, :], e, "3", m, A)
                st[i]["g3"] = g3

            def emit_head(i):
                (e, off, A) = groups[i]
                h = hpool.tile([OUT_DIM, A], F32, tag="h")
                for k in range(NM3[e]):
                    for (ao, aw) in _subs(A):
                        nc.tensor.matmul(
                            h[:, ao:ao + aw],
                            wl_ap(e, 4, k, 0, OUT_DIM, mw=OUT_DIM),
                            st[i]["g3"][:, k, ao:ao + aw],
                            start=(k == 0), stop=(k == NM3[e] - 1))
                ho = hopool.tile([OUT_DIM, A], F32, tag="ho")
                nc.scalar.activation(ho[:], h[:], AF.Identity,
                                     bias=bias_sb[0:OUT_DIM,
                                                  bcols[(e, "bh")]:bcols[(e, "bh")] + 1],
                                     scale=1.0)
                nc.sync.dma_start(out_d[:, off:off + A], ho[:])
                del st[i]

            n = len(groups)
            dma_weights(groups[0][0])
            nc.sync.dma_start(bias_sb[:], bias_d[:])
            for i in range(min(2, n)):
                prefetch(i)
            for e in range(NE):
                if e != groups[0][0]:
                    dma_weights(e)
            for i in range(n + 3):
                if 0 <= i - 3 < n:
                    emit_head(i - 3)
                if 0 <= i - 2 < n:
                    emit_l3(i - 2)
                if 0 <= i - 1 < n:
                    emit_l2(i - 1)
                if i < n:
                    emit_l1(i)
                if i + 2 < n:
                    prefetch(i + 2)

    _legalize_waits(nc)
    return nc


def _prep_weights(W1, b1, W2, b2, W3, b3, Wh, bh):
    """Fold CELU alpha=0.1 scaling and activation shifts into weights."""
    bcols, nbc = _bias_cols()
    wcols, nwc = _w128_cols()
    w126 = np.zeros((KC, NE * NK1 * W1P), np.float32)
    w128 = np.zeros((128, nwc), np.float32)
    bias_pack = np.zeros((128, nbc), np.float32)
    for e, (w1, w2, w3) in enumerate(WIDTHS):
        W1e = 10.0 * np.asarray(W1[e][:, :w1], np.float32)        # [1008, w1]
        b1e = 10.0 * np.asarray(b1[e][:w1], np.float32)
        W2e = np.asarray(W2[e][:w1, :w2], np.float32)
        b2e = 10.0 * np.asarray(b2[e][:w2], np.float32)
        W3e = np.asarray(W3[e][:w2, :w3], np.float32)
        b3e = 10.0 * np.asarray(b3[e][:w3], np.float32)
        Whe = 0.1 * np.asarray(Wh[e][:w3, :], np.float32)
        bhe = np.asarray(bh[e], np.float32)

        W1p = np.zeros((AEV, W1P), np.float32); W1p[:, :w1] = W1e
        W2p = np.zeros((W1P, W2P[e]), np.float32); W2p[:w1, :w2] = W2e
        W3p = np.zeros((W2P[e], W3P[e]), np.float32); W3p[:w2, :w3] = W3e
        Whp = np.zeros((W3P[e], OUT_DIM), np.float32); Whp[:w3, :] = Whe

        beta1 = np.zeros(W1P, np.float32); beta1[:w1] = b1e
        beta2 = np.zeros(W2P[e], np.float32); beta2[:w2] = b2e
        beta2 += beta1 @ W2p
        beta3 = np.zeros(W3P[e], np.float32); beta3[:w3] = b3e
        beta3 += beta2 @ W3p
        bh_eff = bhe + beta3 @ Whp

        # device layouts: W1 -> [126, k, m*128] blocks; W2/W3/Wh -> packed cols
        blk = W1p.reshape(NK1, KC, W1P).transpose(1, 0, 2).reshape(KC, NK1 * W1P)
        w126[:, e * NK1 * W1P:(e + 1) * NK1 * W1P] = blk

        def pack(l, Wp, nk):
            c0 = wcols[(e, l)]
            wp = Wp.shape[1]
            blk = Wp.reshape(nk, 128, wp).transpose(1, 0, 2).reshape(128, nk * wp)
            w128[:, c0:c0 + nk * wp] = blk

        pack(2, W2p, NM1)
        pack(3, W3p, NM2[e])
        pack(4, Whp, NM3[e])

        def put(key, vec, nm):
            c0 = bcols[(e, key)]
            for m in range(nm):
                seg = vec[m * 128:(m + 1) * 128]
                bias_pack[:len(seg), c0 + m] = seg

        put("b1", beta1, NM1); put("n1", -(1.0 + beta1), NM1)
        put("b2", beta2, NM2[e]); put("n2", -(1.0 + beta2), NM2[e])
        put("b3", beta3, NM3[e]); put("n3", -(1.0 + beta3), NM3[e])
        bias_pack[0:OUT_DIM, bcols[(e, "bh")]] = bh_eff
    return w126.astype(BF16_NP), w128.astype(BF16_NP), bias_pack


def kernel(**inputs):
    species = np.asarray(inputs["species"]).astype(np.int64).ravel()
    aev = np.asarray(inputs["aev"], dtype=np.float32)
    N = species.shape[0]

    order = np.argsort(species, kind="stable")
    counts = np.bincount(species, minlength=NE)

    # per-core per-species share, rounded up to a multiple of 4
    share = [((-(-int(counts[e]) // N_CORES)) + 3) // 4 * 4 for e in range(NE)]
    per_core_n = sum(share)

    idx = np.full((N_CORES, per_core_n), N, dtype=np.int64)
    groups = []
    off = 0
    start = 0
    for e in range(NE):
        ids = order[start:start + counts[e]]
        start += counts[e]
        s = share[e]
        if s == 0:
            continue
        buf = np.full(N_CORES * s, N, dtype=np.int64)
        buf[:len(ids)] = ids
        idx[:, off:off + s] = buf.reshape(N_CORES, s)
        o = 0
        while o < s:
            A = min(GROUP_ATOMS, s - o)
            groups.append((e, off + o, A))
            o += A
        off += s

    w126, w128, bias_pack = _prep_weights(
        inputs["W1"], inputs["b1"], inputs["W2"], inputs["b2"],
        inputs["W3"], inputs["b3"], inputs["Wh"], inputs["bh"])

    aev_ext = np.concatenate([aev, np.zeros((1, AEV), np.float32)], axis=0)

    in_maps = []
    for c in range(N_CORES):
        A_c = aev_ext[idx[c]].astype(BF16_NP)             # [per_core_n, 1008]
        A_t = A_c.T.reshape(NK1, KC, per_core_n)          # [k, p, atom]
        X = np.empty((KC, NK1 * per_core_n), BF16_NP)
        for (e, off, A) in groups:
            seg = A_t[:, :, off:off + A].transpose(1, 0, 2)   # [126, 8, A]
            X[:, NK1 * off:NK1 * (off + A)] = seg.reshape(KC, NK1 * A)
        in_maps.append({"aev": X, "w126": w126, "w128": w128, "bias": bias_pack})

    nc = _build(groups, per_core_n)
    res = run_bass_kernel_spmd(nc, in_maps, core_ids=list(range(N_CORES)))

    out_full = np.zeros((N, OUT_DIM), np.float32)
    for c in range(N_CORES):
        o = np.asarray(res.results[c]["out"])             # [2, per_core_n]
        mask = idx[c] < N
        out_full[idx[c][mask]] = o.T[mask]
    return out_full


# revision 10
# speedup vs baseline: 4.8629x; 4.8629x over previous
"""ANI-style species-routed MoE MLP on 8 Trainium2 NeuronCores.

Strategy:
- Host routing: sort atoms by species (stable argsort), shard each species'
  atoms evenly across 8 cores.  Each core runs ONE expert per atom instead
  of all 7 (the reference's dense masking wastes 7x compute).
- Transposed dataflow: activations live as [feature, atom] so the 3-layer
  MLP chains matmuls without transposes (lhsT = weights stay stationary).
- bf16 matmul inputs, fp32 PSUM accumulation.
- CELU(z, a=0.1) with all scale factors folded into the weights host-side:
  internal activations are G = 10*celu(z) - beta (shifted); per layer
      p    = G_prev @ W            (PSUM, fp32)
      e    = exp(p + beta)         (ScalarE, bias folded)
      u    = min(e,1) + (-1-beta)  (VectorE tensor_scalar, dual-op)
      G    = max(p, u)             (VectorE tensor_tensor)
  where beta absorbs the layer bias AND the shift of the previous layer.
- DMA layout: per-core aev is stored group-contiguous ([126, 8*A] blocks)
  so each group loads with 126 x 16KB descriptors instead of ~1000 x 2KB.
"""

import math

import numpy as np
import ml_dtypes

import concourse.bass as bass
import concourse.mybir as mybir
from concourse import tile
from concourse.bass_utils import run_bass_kernel_spmd

AF = mybir.ActivationFunctionType
ALU = mybir.AluOpType
BF16 = mybir.dt.bfloat16
F32 = mybir.dt.float32
BF16_NP = ml_dtypes.bfloat16

N_CORES = 8
AEV = 1008
KC = 126          # L1 contraction chunk rows (8 * 126 = 1008)
NK1 = 8
OUT_DIM = 2
GROUP_ATOMS = 512

WIDTHS = [(256, 192, 160), (224, 192, 160), (192, 160, 128), (192, 160, 128),
          (160, 128, 96), (160, 128, 96), (160, 128, 96)]
NE = len(WIDTHS)

NM1 = 2                       # w1 padded to 256 for every expert
W1P = NM1 * 128
NM2 = [math.ceil(w2 / 128) for (_, w2, _) in WIDTHS]   # [2,2,2,2,1,1,1]
NM3 = [math.ceil(w3 / 128) for (_, _, w3) in WIDTHS]   # [2,2,1,1,1,1,1]
W2P = [m * 128 for m in NM2]
W3P = [m * 128 for m in NM3]


def _bias_cols():
    cols = {}
    c = 0
    for e in range(NE):
        cols[(e, "b1")] = c; c += NM1
        cols[(e, "n1")] = c; c += NM1
        cols[(e, "b2")] = c; c += NM2[e]
        cols[(e, "n2")] = c; c += NM2[e]
        cols[(e, "b3")] = c; c += NM3[e]
        cols[(e, "n3")] = c; c += NM3[e]
        cols[(e, "bh")] = c; c += 1
    return cols, c


def _w128_cols():
    """Column offsets inside the packed [128, C] weight tensor holding
    W2/W3/Wh blocks for every expert (K-chunk-major per tensor)."""
    cols = {}
    c = 0
    for e in range(NE):
        cols[(e, 2)] = c; c += NM1 * W2P[e]
        cols[(e, 3)] = c; c += NM2[e] * W3P[e]
        cols[(e, 4)] = c; c += NM3[e] * OUT_DIM
    return cols, c


def _legalize_waits(nc, limit=1):
    """This container's walrus accepts at most one sync-wait per
    instruction; split extras into standalone NoOps on the same engine."""
    for fn in nc.m.functions:
        for blk in fn.blocks:
            out, changed = [], False
            for inst in blk.instructions:
                si = inst.sync_info
                waits = list(si.on_wait) if si is not None and si.on_wait else []
                if len(waits) > limit:
                    changed = True
                    for k, w in enumerate(waits[:-limit]):
                        nop = mybir.InstNoOp(name=f"{inst.name}_lw{k}", ins=[], outs=[])
                        nop.engine = inst.engine
                        nop.sync_info = mybir.SyncInfo(on_wait=[w], on_update=[])
                        out.append(nop)
                    upd = list(si.on_update) if si.on_update else []
                    inst.sync_info = mybir.SyncInfo(on_wait=waits[-limit:], on_update=upd)
                out.append(inst)
            if changed:
                blk.instructions = out


def _subs(A):
    return [(o, min(512, A - o)) for o in range(0, A, 512)]


def _build(groups, per_core_n):
    bcols, nbc = _bias_cols()
    wcols, nwc = _w128_cols()
    nc = bass.Bass(trn_type="TRN2")

    aev_d = nc.dram_tensor("aev", (KC, NK1 * per_core_n), BF16, kind="ExternalInput")
    w126_d = nc.dram_tensor("w126", (KC, NE * NK1 * W1P), BF16, kind="ExternalInput")
    w128_d = nc.dram_tensor("w128", (128, nwc), BF16, kind="ExternalInput")
    bias_d = nc.dram_tensor("bias", (128, nbc), F32, kind="ExternalInput")
    out_d = nc.dram_tensor("out", (OUT_DIM, per_core_n), F32, kind="ExternalOutput")

    with tile.TileContext(nc) as tc:
        with (
            tc.tile_pool(name="wpool", bufs=1) as wpool,
            tc.tile_pool(name="xpool", bufs=4) as xpool,
            tc.tile_pool(name="gpool", bufs=3) as gpool,
            tc.tile_pool(name="tpool", bufs=4) as tpool,
            tc.tile_pool(name="hopool", bufs=3) as hopool,
            tc.tile_pool(name="zpool", bufs=6, space="PSUM") as zpool,
            tc.tile_pool(name="hpool", bufs=2, space="PSUM") as hpool,
        ):
            w126_sb = wpool.tile([KC, NE * NK1 * W1P], BF16, tag="w126")
            w128_sb = wpool.tile([128, nwc], BF16, tag="w128")
            bias_sb = wpool.tile([128, nbc], F32, tag="bias")

            def dma_weights(e):
                c0, c1 = e * NK1 * W1P, (e + 1) * NK1 * W1P
                nc.sync.dma_start(w126_sb[:, c0:c1], w126_d[:, c0:c1])
                d0 = wcols[(e, 2)]
                d1 = wcols[(e, 4)] + NM3[e] * OUT_DIM
                nc.sync.dma_start(w128_sb[:, d0:d1], w128_d[:, d0:d1])

            def w1_ap(e, k, m):
                c = e * NK1 * W1P + k * W1P + m * 128
                return w126_sb[:, c:c + 128]

            def wl_ap(e, l, k, m, wp, mw=128):
                c = wcols[(e, l)] + k * wp + m * 128
                return w128_sb[:, c:c + mw]

            def bcol(e, key, m):
                c = bcols[(e, key)] + m
                return bias_sb[:, c:c + 1]

            def celu(z, g_out, e_, key, m, A):
                # e = exp(z + beta); u = min(e,1) + (-1-beta); G = max(z, u)
                e_t = tpool.tile([128, A], BF16, tag="e")
                nc.scalar.activation(e_t[:], z[:], AF.Exp,
                                     bias=bcol(e_, "b" + key, m), scale=1.0)
                u_t = tpool.tile([128, A], BF16, tag="u")
                nc.vector.tensor_scalar(u_t[:], e_t[:], 1.0,
                                        bcol(e_, "n" + key, m), ALU.min, ALU.add)
                nc.vector.tensor_tensor(g_out[:], z[:], u_t[:], ALU.max)

            # Software pipeline: PE stream interleaves stages of different
            # groups so each group's celu chain (EXP->MIN,ADD->MAX) is
            # hidden behind another group's matmuls.
            st = {}   # group idx -> dict(x, g1, g2, g3)

            def prefetch(i):
                (e, off, A) = groups[i]
                x = xpool.tile([KC, NK1, A], BF16, tag="x")
                nc.sync.dma_start(x[:], aev_d[:, NK1 * off:NK1 * (off + A)]
                                  .rearrange("p (k a) -> p k a", k=NK1))
                st[i] = {"x": x}

            def emit_l1(i):
                (e, off, A) = groups[i]
                g1 = gpool.tile([128, NM1, A], BF16, tag="g1")
                for m in range(NM1):
                    z = zpool.tile([128, A], F32, tag="z")
                    for k in range(NK1):
                        for (ao, aw) in _subs(A):
                            nc.tensor.matmul(
                                z[:, ao:ao + aw], w1_ap(e, k, m),
                                st[i]["x"][:, k, ao:ao + aw],
                                start=(k == 0), stop=(k == NK1 - 1))
                    celu(z, g1[:, m, :], e, "1", m, A)
                st[i]["g1"] = g1

            def emit_l2(i):
                (e, off, A) = groups[i]
                g2 = gpool.tile([128, NM2[e], A], BF16, tag="g2")
                for m in range(NM2[e]):
                    z = zpool.tile([128, A], F32, tag="z")
                    for k in range(NM1):
                        for (ao, aw) in _subs(A):
                            nc.tensor.matmul(
                                z[:, ao:ao + aw], wl_ap(e, 2, k, m, W2P[e]),
                                st[i]["g1"][:, k, ao:ao + aw],
                                start=(k == 0), stop=(k == NM1 - 1))
                    celu(z, g2[:, m, :], e, "2", m, A)
                st[i]["g2"] = g2

            def emit_l3(i):
                (e, off, A) = groups[i]
                g3 = gpool.tile([128, NM3[e], A], BF16, tag="g3")
                for m in range(NM3[e]):
                    z = zpool.tile([128, A], F32, tag="z")
                    for k in range(NM2[e]):
                        for (ao, aw) in _subs(A):
                            nc.tensor.matmul(
                                z[:, ao:ao + aw], wl_ap(e, 3, k, m, W3P[e]),
                                st[i]["g2"][:, k, ao:ao + aw],
                                start=(k == 0), stop=(k == NM2[e] - 1))
                    celu(z, g3[:, m, :], e, "3", m, A)
                st[i]["g3"] = g3

            def emit_head(i):
                (e, off, A) = groups[i]
                h = hpool.tile([OUT_DIM, A], F32, tag="h")
                for k in range(NM3[e]):
                    for (ao, aw) in _subs(A):
                        nc.tensor.matmul(
                            h[:, ao:ao + aw],
                            wl_ap(e, 4, k, 0, OUT_DIM, mw=OUT_DIM),
                            st[i]["g3"][:, k, ao:ao + aw],
                            start=(k == 0), stop=(k == NM3[e] - 1))
                ho = hopool.tile([OUT_DIM, A], F32, tag="ho")
                nc.scalar.activation(ho[:], h[:], AF.Identity,
                                     bias=bias_sb[0:OUT_DIM,
                                                  bcols[(e, "bh")]:bcols[(e, "bh")] + 1],
                                     scale=1.0)
                nc.sync.dma_start(out_d[:, off:off + A], ho[:])
                del st[i]

            n = len(groups)
            dma_weights(groups[0][0])
            nc.sync.dma_start(bias_sb[:], bias_d[:])
            for i in range(min(2, n)):
                prefetch(i)
            for e in range(NE):
                if e != groups[0][0]:
                    dma_weights(e)
            for i in range(n + 3):
                if 0 <= i - 3 < n:
                    emit_head(i - 3)
                if 0 <= i - 2 < n:
                    emit_l3(i - 2)
                if 0 <= i - 1 < n:
                    emit_l2(i - 1)
                if i < n:
                    emit_l1(i)
                if i + 2 < n:
                    prefetch(i + 2)

    _legalize_waits(nc)
    return nc


def _prep_weights(W1, b1, W2, b2, W3, b3, Wh, bh):
    """Fold CELU alpha=0.1 scaling and activation shifts into weights."""
    bcols, nbc = _bias_cols()
    wcols, nwc = _w128_cols()
    w126 = np.zeros((KC, NE * NK1 * W1P), np.float32)
    w128 = np.zeros((128, nwc), np.float32)
    bias_pack = np.zeros((128, nbc), np.float32)
    for e, (w1, w2, w3) in enumerate(WIDTHS):
        W1e = 10.0 * np.asarray(W1[e][:, :w1], np.float32)        # [1008, w1]
        b1e = 10.0 * np.asarray(b1[e][:w1], np.float32)
        W2e = np.asarray(W2[e][:w1, :w2], np.float32)
        b2e = 10.0 * np.asarray(b2[e][:w2], np.float32)
        W3e = np.asarray(W3[e][:w2, :w3], np.float32)
        b3e = 10.0 * np.asarray(b3[e][:w3], np.float32)
        Whe = 0.1 * np.asarray(Wh[e][:w3, :], np.float32)
        bhe = np.asarray(bh[e], np.float32)

        W1p = np.zeros((AEV, W1P), np.float32); W1p[:, :w1] = W1e
        W2p = np.zeros((W1P, W2P[e]), np.float32); W2p[:w1, :w2] = W2e
        W3p = np.zeros((W2P[e], W3P[e]), np.float32); W3p[:w2, :w3] = W3e
        Whp = np.zeros((W3P[e], OUT_DIM), np.float32); Whp[:w3, :] = Whe

        beta1 = np.zeros(W1P, np.float32); beta1[:w1] = b1e
        beta2 = np.zeros(W2P[e], np.float32); beta2[:w2] = b2e
        beta2 += beta1 @ W2p
        beta3 = np.zeros(W3P[e], np.float32); beta3[:w3] = b3e
        beta3 += beta2 @ W3p
        bh_eff = bhe + beta3 @ Whp

        # device layouts: W1 -> [126, k, m*128] blocks; W2/W3/Wh -> packed cols
        blk = W1p.reshape(NK1, KC, W1P).transpose(1, 0, 2).reshape(KC, NK1 * W1P)
        w126[:, e * NK1 * W1P:(e + 1) * NK1 * W1P] = blk

        def pack(l, Wp, nk):
            c0 = wcols[(e, l)]
            wp = Wp.shape[1]
            blk = Wp.reshape(nk, 128, wp).transpose(1, 0, 2).reshape(128, nk * wp)
            w128[:, c0:c0 + nk * wp] = blk

        pack(2, W2p, NM1)
        pack(3, W3p, NM2[e])
        pack(4, Whp, NM3[e])

        def put(key, vec, nm):
            c0 = bcols[(e, key)]
            for m in range(nm):
                seg = vec[m * 128:(m + 1) * 128]
                bias_pack[:len(seg), c0 + m] = seg

        put("b1", beta1, NM1); put("n1", -(1.0 + beta1), NM1)
        put("b2", beta2, NM2[e]); put("n2", -(1.0 + beta2), NM2[e])
        put("b3", beta3, NM3[e]); put("n3", -(1.0 + beta3), NM3[e])
        bias_pack[0:OUT_DIM, bcols[(e, "bh")]] = bh_eff
    return w126.astype(BF16_NP), w128.astype(BF16_NP), bias_pack


def kernel(**inputs):
    species = np.asarray(inputs["species"]).astype(np.int64).ravel()
    aev = np.asarray(inputs["aev"], dtype=np.float32)
    N = species.shape[0]

    order = np.argsort(species, kind="stable")
    counts = np.bincount(species, minlength=NE)

    # per-core per-species share, rounded up to a multiple of 4
    share = [((-(-int(counts[e]) // N_CORES)) + 3) // 4 * 4 for e in range(NE)]
    per_core_n = sum(share)

    idx = np.full((N_CORES, per_core_n), N, dtype=np.int64)
    groups = []
    off = 0
    start = 0
    for e in range(NE):
        ids = order[start:start + counts[e]]
        start += counts[e]
        s = share[e]
        if s == 0:
            continue
        buf = np.full(N_CORES * s, N, dtype=np.int64)
        buf[:len(ids)] = ids
        idx[:, off:off + s] = buf.reshape(N_CORES, s)
        o = 0
        while o < s:
            A = min(GROUP_ATOMS, s - o)
            groups.append((e, off + o, A))
            o += A
        off += s

    w126, w128, bias_pack = _prep_weights(
        inputs["W1"], inputs["b1"], inputs["W2"], inputs["b2"],
        inputs["W3"], inputs["b3"], inputs["Wh"], inputs["bh"])

    aev_ext = np.concatenate([aev, np.zeros((1, AEV), np.float32)], axis=0)

    in_maps = []
    for c in range(N_CORES):
        A_c = aev_ext[idx[c]].astype(BF16_NP)             # [per_core_n, 1008]
        A_t = A_c.T.reshape(NK1, KC, per_core_n)          # [k, p, atom]
        X = np.empty((KC, NK1 * per_core_n), BF16_NP)
        for (e, off, A) in groups:
            seg = A_t[:, :, off:off + A].transpose(1, 0, 2)   # [126, 8, A]
            X[:, NK1 * off:NK1 * (off + A)] = seg.reshape(KC, NK1 * A)
        in_maps.append({"aev": X, "w126": w126, "w128": w128, "bias": bias_pack})

    nc = _build(groups, per_core_n)
    res = run_bass_kernel_spmd(nc, in_maps, core_ids=list(range(N_CORES)))

    out_full = np.zeros((N, OUT_DIM), np.float32)
    for c in range(N_CORES):
        o = np.asarray(res.results[c]["out"])             # [2, per_core_n]
        mask = idx[c] < N
        out_full[idx[c][mask]] = o.T[mask]
    return out_full


# revision 11
# speedup vs baseline: 5.4931x; 1.1296x over previous
"""ANI-style species-routed MoE MLP on 8 Trainium2 NeuronCores.

Strategy:
- Host routing: sort atoms by species (stable argsort), shard each species'
  atoms evenly across 8 cores.  Each core runs ONE expert per atom instead
  of all 7 (the reference's dense masking wastes 7x compute).
- Transposed dataflow: activations live as [feature, atom] so the 3-layer
  MLP chains matmuls without transposes (lhsT = weights stay stationary).
- bf16 matmul inputs, fp32 PSUM accumulation.
- CELU(z, a=0.1) with all scale factors folded into the weights host-side:
  internal activations are G = 10*celu(z) - beta (shifted); per layer
      p    = G_prev @ W            (PSUM, fp32)
      e    = exp(p + beta)         (ScalarE, bias folded)
      u    = min(e,1) + (-1-beta)  (VectorE tensor_scalar, dual-op)
      G    = max(p, u)             (VectorE tensor_tensor)
  where beta absorbs the layer bias AND the shift of the previous layer.
- DMA layout: per-core aev is stored group-contiguous ([126, 8*A] blocks)
  so each group loads with 126 x 16KB descriptors instead of ~1000 x 2KB.
"""

import math

import numpy as np
import ml_dtypes

import concourse.bass as bass
import concourse.mybir as mybir
from concourse import tile
from concourse.bass_utils import run_bass_kernel_spmd

AF = mybir.ActivationFunctionType
ALU = mybir.AluOpType
BF16 = mybir.dt.bfloat16
F32 = mybir.dt.float32
BF16_NP = ml_dtypes.bfloat16

N_CORES = 8
AEV = 1008
KC = 126          # L1 contraction chunk rows (8 * 126 = 1008)
NK1 = 8
OUT_DIM = 2
GROUP_ATOMS = 512

WIDTHS = [(256, 192, 160), (224, 192, 160), (192, 160, 128), (192, 160, 128),
          (160, 128, 96), (160, 128, 96), (160, 128, 96)]
NE = len(WIDTHS)

NM1 = 2                       # w1 padded to 256 for every expert
W1P = NM1 * 128
NM2 = [math.ceil(w2 / 128) for (_, w2, _) in WIDTHS]   # [2,2,2,2,1,1,1]
NM3 = [math.ceil(w3 / 128) for (_, _, w3) in WIDTHS]   # [2,2,1,1,1,1,1]
W2P = [m * 128 for m in NM2]
W3P = [m * 128 for m in NM3]


def _bias_cols():
    cols = {}
    c = 0
    for e in range(NE):
        cols[(e, "b1")] = c; c += NM1
        cols[(e, "n1")] = c; c += NM1
        cols[(e, "b2")] = c; c += NM2[e]
        cols[(e, "n2")] = c; c += NM2[e]
        cols[(e, "b3")] = c; c += NM3[e]
        cols[(e, "n3")] = c; c += NM3[e]
        cols[(e, "bh")] = c; c += 1
    return cols, c


def _w128_cols():
    """Column offsets inside the packed [128, C] weight tensor holding
    W2/W3/Wh blocks for every expert (K-chunk-major per tensor)."""
    cols = {}
    c = 0
    for e in range(NE):
        cols[(e, 2)] = c; c += NM1 * W2P[e]
        cols[(e, 3)] = c; c += NM2[e] * W3P[e]
        cols[(e, 4)] = c; c += NM3[e] * OUT_DIM
    return cols, c


def _legalize_waits(nc, limit=1):
    """This container's walrus accepts at most one sync-wait per
    instruction; split extras into standalone NoOps on the same engine."""
    for fn in nc.m.functions:
        for blk in fn.blocks:
            out, changed = [], False
            for inst in blk.instructions:
                si = inst.sync_info
                waits = list(si.on_wait) if si is not None and si.on_wait else []
                if len(waits) > limit:
                    changed = True
                    for k, w in enumerate(waits[:-limit]):
                        nop = mybir.InstNoOp(name=f"{inst.name}_lw{k}", ins=[], outs=[])
                        nop.engine = inst.engine
                        nop.sync_info = mybir.SyncInfo(on_wait=[w], on_update=[])
                        out.append(nop)
                    upd = list(si.on_update) if si.on_update else []
                    inst.sync_info = mybir.SyncInfo(on_wait=waits[-limit:], on_update=upd)
                out.append(inst)
            if changed:
                blk.instructions = out


def _subs(A):
    return [(o, min(512, A - o)) for o in range(0, A, 512)]


def _build(groups, per_core_n):
    bcols, nbc = _bias_cols()
    wcols, nwc = _w128_cols()
    nc = bass.Bass(trn_type="TRN2")

    aev_d = nc.dram_tensor("aev", (KC, NK1 * per_core_n), BF16, kind="ExternalInput")
    w126_d = nc.dram_tensor("w126", (KC, NE * NK1 * W1P), BF16, kind="ExternalInput")
    w128_d = nc.dram_tensor("w128", (128, nwc), BF16, kind="ExternalInput")
    bias_d = nc.dram_tensor("bias", (128, nbc), F32, kind="ExternalInput")
    out_d = nc.dram_tensor("out", (OUT_DIM, per_core_n), F32, kind="ExternalOutput")

    with tile.TileContext(nc) as tc:
        with (
            tc.tile_pool(name="wpool", bufs=1) as wpool,
            tc.tile_pool(name="xpool", bufs=4) as xpool,
            tc.tile_pool(name="gpool", bufs=3) as gpool,
            tc.tile_pool(name="tpool", bufs=4) as tpool,
            tc.tile_pool(name="hopool", bufs=3) as hopool,
            tc.tile_pool(name="zpool", bufs=6, space="PSUM") as zpool,
            tc.tile_pool(name="hpool", bufs=2, space="PSUM") as hpool,
        ):
            w126_sb = wpool.tile([KC, NE * NK1 * W1P], BF16, tag="w126")
            w128_sb = wpool.tile([128, nwc], BF16, tag="w128")
            bias_sb = wpool.tile([128, nbc], F32, tag="bias")

            def dma_weights(e):
                c0, c1 = e * NK1 * W1P, (e + 1) * NK1 * W1P
                nc.sync.dma_start(w126_sb[:, c0:c1], w126_d[:, c0:c1])
                d0 = wcols[(e, 2)]
                d1 = wcols[(e, 4)] + NM3[e] * OUT_DIM
                nc.sync.dma_start(w128_sb[:, d0:d1], w128_d[:, d0:d1])

            def w1_ap(e, k, m):
                c = e * NK1 * W1P + k * W1P + m * 128
                return w126_sb[:, c:c + 128]

            def wl_ap(e, l, k, m, wp, mw=128):
                c = wcols[(e, l)] + k * wp + m * 128
                return w128_sb[:, c:c + mw]

            def bcol(e, key, m):
                c = bcols[(e, key)] + m
                return bias_sb[:, c:c + 1]

            def celu(z, g_out, e_, key, m, A):
                # e = exp(z + beta); u = min(e,1) + (-1-beta); G = max(z, u)
                e_t = tpool.tile([128, A], BF16, tag="e")
                nc.scalar.activation(e_t[:], z[:], AF.Exp,
                                     bias=bcol(e_, "b" + key, m), scale=1.0)
                u_t = tpool.tile([128, A], BF16, tag="u")
                nc.vector.tensor_scalar(u_t[:], e_t[:], 1.0,
                                        bcol(e_, "n" + key, m), ALU.min, ALU.add)
                nc.vector.tensor_tensor(g_out[:], z[:], u_t[:], ALU.max)

            # Software pipeline: PE stream interleaves stages of different
            # groups so each group's celu chain (EXP->MIN,ADD->MAX) is
            # hidden behind another group's matmuls.
            st = {}   # group idx -> dict(x, g1, g2, g3)

            def prefetch(i):
                (e, off, A) = groups[i]
                x = xpool.tile([KC, NK1, A], BF16, tag="x")
                nc.sync.dma_start(x[:], aev_d[:, NK1 * off:NK1 * (off + A)]
                                  .rearrange("p (k a) -> p k a", k=NK1))
                st[i] = {"x": x}

            def emit_l1(i):
                (e, off, A) = groups[i]
                g1 = gpool.tile([128, NM1, A], BF16, tag="g1")
                for m in range(NM1):
                    z = zpool.tile([128, A], F32, tag="z")
                    for k in range(NK1):
                        for (ao, aw) in _subs(A):
                            nc.tensor.matmul(
                                z[:, ao:ao + aw], w1_ap(e, k, m),
                                st[i]["x"][:, k, ao:ao + aw],
                                start=(k == 0), stop=(k == NK1 - 1))
                    celu(z, g1[:, m, :], e, "1", m, A)
                st[i]["g1"] = g1

            def emit_l2(i):
                (e, off, A) = groups[i]
                g2 = gpool.tile([128, NM2[e], A], BF16, tag="g2")
                for m in range(NM2[e]):
                    z = zpool.tile([128, A], F32, tag="z")
                    for k in range(NM1):
                        for (ao, aw) in _subs(A):
                            nc.tensor.matmul(
                                z[:, ao:ao + aw], wl_ap(e, 2, k, m, W2P[e]),
                                st[i]["g1"][:, k, ao:ao + aw],
                                start=(k == 0), stop=(k == NM1 - 1))
                    celu(z, g2[:, m, :], e, "2", m, A)
                st[i]["g2"] = g2

            def emit_l3(i):
                (e, off, A) = groups[i]
                g3 = gpool.tile([128, NM3[e], A], BF16, tag="g3")
                for m in range(NM3[e]):
                    z = zpool.tile([128, A], F32, tag="z")
                    for k in range(NM2[e]):
                        for (ao, aw) in _subs(A):
                            nc.tensor.matmul(
                                z[:, ao:ao + aw], wl_ap(e, 3, k, m, W3P[e]),
                                st[i]["g2"][:, k, ao:ao + aw],
                                start=(k == 0), stop=(k == NM2[e] - 1))
                    celu(z, g3[:, m, :], e, "3", m, A)
                st[i]["g3"] = g3

            def emit_head(i):
                (e, off, A) = groups[i]
                h = hpool.tile([OUT_DIM, A], F32, tag="h")
                for k in range(NM3[e]):
                    for (ao, aw) in _subs(A):
                        nc.tensor.matmul(
                            h[:, ao:ao + aw],
                            wl_ap(e, 4, k, 0, OUT_DIM, mw=OUT_DIM),
                            st[i]["g3"][:, k, ao:ao + aw],
                            start=(k == 0), stop=(k == NM3[e] - 1))
                ho = hopool.tile([OUT_DIM, A], F32, tag="ho")
                nc.scalar.activation(ho[:], h[:], AF.Identity,
                                     bias=bias_sb[0:OUT_DIM,
                                                  bcols[(e, "bh")]:bcols[(e, "bh")] + 1],
                                     scale=1.0)
                nc.sync.dma_start(out_d[:, off:off + A], ho[:])
                del st[i]

            n = len(groups)
            for e in range(NE):
                dma_weights(e)
            nc.sync.dma_start(bias_sb[:], bias_d[:])
            for i in range(min(2, n)):
                prefetch(i)
            for i in range(n + 3):
                if 0 <= i - 3 < n:
                    emit_head(i - 3)
                if 0 <= i - 2 < n:
                    emit_l3(i - 2)
                if 0 <= i - 1 < n:
                    emit_l2(i - 1)
                if i < n:
                    emit_l1(i)
                if i + 2 < n:
                    prefetch(i + 2)

    _legalize_waits(nc)
    return nc


def _prep_weights(W1, b1, W2, b2, W3, b3, Wh, bh):
    """Fold CELU alpha=0.1 scaling and activation shifts into weights."""
    bcols, nbc = _bias_cols()
    wcols, nwc = _w128_cols()
    w126 = np.zeros((KC, NE * NK1 * W1P), np.float32)
    w128 = np.zeros((128, nwc), np.float32)
    bias_pack = np.zeros((128, nbc), np.float32)
    for e, (w1, w2, w3) in enumerate(WIDTHS):
        W1e = 10.0 * np.asarray(W1[e][:, :w1], np.float32)        # [1008, w1]
        b1e = 10.0 * np.asarray(b1[e][:w1], np.float32)
        W2e = np.asarray(W2[e][:w1, :w2], np.float32)
        b2e = 10.0 * np.asarray(b2[e][:w2], np.float32)
        W3e = np.asarray(W3[e][:w2, :w3], np.float32)
        b3e = 10.0 * np.asarray(b3[e][:w3], np.float32)
        Whe = 0.1 * np.asarray(Wh[e][:w3, :], np.float32)
        bhe = np.asarray(bh[e], np.float32)

        W1p = np.zeros((AEV, W1P), np.float32); W1p[:, :w1] = W1e
        W2p = np.zeros((W1P, W2P[e]), np.float32); W2p[:w1, :w2] = W2e
        W3p = np.zeros((W2P[e], W3P[e]), np.float32); W3p[:w2, :w3] = W3e
        Whp = np.zeros((W3P[e], OUT_DIM), np.float32); Whp[:w3, :] = Whe

        beta1 = np.zeros(W1P, np.float32); beta1[:w1] = b1e
        beta2 = np.zeros(W2P[e], np.float32); beta2[:w2] = b2e
        beta2 += beta1 @ W2p
        beta3 = np.zeros(W3P[e], np.float32); beta3[:w3] = b3e
        beta3 += beta2 @ W3p
        bh_eff = bhe + beta3 @ Whp

        # device layouts: W1 -> [126, k, m*128] blocks; W2/W3/Wh -> packed cols
        blk = W1p.reshape(NK1, KC, W1P).transpose(1, 0, 2).reshape(KC, NK1 * W1P)
        w126[:, e * NK1 * W1P:(e + 1) * NK1 * W1P] = blk

        def pack(l, Wp, nk):
            c0 = wcols[(e, l)]
            wp = Wp.shape[1]
            blk = Wp.reshape(nk, 128, wp).transpose(1, 0, 2).reshape(128, nk * wp)
            w128[:, c0:c0 + nk * wp] = blk

        pack(2, W2p, NM1)
        pack(3, W3p, NM2[e])
        pack(4, Whp, NM3[e])

        def put(key, vec, nm):
            c0 = bcols[(e, key)]
            for m in range(nm):
                seg = vec[m * 128:(m + 1) * 128]
                bias_pack[:len(seg), c0 + m] = seg

        put("b1", beta1, NM1); put("n1", -(1.0 + beta1), NM1)
        put("b2", beta2, NM2[e]); put("n2", -(1.0 + beta2), NM2[e])
        put("b3", beta3, NM3[e]); put("n3", -(1.0 + beta3), NM3[e])
        bias_pack[0:OUT_DIM, bcols[(e, "bh")]] = bh_eff
    return w126.astype(BF16_NP), w128.astype(BF16_NP), bias_pack


def kernel(**inputs):
    species = np.asarray(inputs["species"]).astype(np.int64).ravel()
    aev = np.asarray(inputs["aev"], dtype=np.float32)
    N = species.shape[0]

    order = np.argsort(species, kind="stable")
    counts = np.bincount(species, minlength=NE)

    # per-core per-species share, rounded up to a multiple of 4
    share = [((-(-int(counts[e]) // N_CORES)) + 3) // 4 * 4 for e in range(NE)]
    per_core_n = sum(share)

    idx = np.full((N_CORES, per_core_n), N, dtype=np.int64)
    groups = []
    off = 0
    start = 0
    for e in range(NE):
        ids = order[start:start + counts[e]]
        start += counts[e]
        s = share[e]
        if s == 0:
            continue
        buf = np.full(N_CORES * s, N, dtype=np.int64)
        buf[:len(ids)] = ids
        idx[:, off:off + s] = buf.reshape(N_CORES, s)
        o = 0
        while o < s:
            A = min(GROUP_ATOMS, s - o)
            groups.append((e, off + o, A))
            o += A
        off += s

    w126, w128, bias_pack = _prep_weights(
        inputs["W1"], inputs["b1"], inputs["W2"], inputs["b2"],
        inputs["W3"], inputs["b3"], inputs["Wh"], inputs["bh"])

    aev_ext = np.concatenate([aev, np.zeros((1, AEV), np.float32)], axis=0)

    in_maps = []
    for c in range(N_CORES):
        A_c = aev_ext[idx[c]].astype(BF16_NP)             # [per_core_n, 1008]
        A_t = A_c.T.reshape(NK1, KC, per_core_n)          # [k, p, atom]
        X = np.empty((KC, NK1 * per_core_n), BF16_NP)
        for (e, off, A) in groups:
            seg = A_t[:, :, off:off + A].transpose(1, 0, 2)   # [126, 8, A]
            X[:, NK1 * off:NK1 * (off + A)] = seg.reshape(KC, NK1 * A)
        in_maps.append({"aev": X, "w126": w126, "w128": w128, "bias": bias_pack})

    nc = _build(groups, per_core_n)
    res = run_bass_kernel_spmd(nc, in_maps, core_ids=list(range(N_CORES)))

    out_full = np.zeros((N, OUT_DIM), np.float32)
    for c in range(N_CORES):
        o = np.asarray(res.results[c]["out"])             # [2, per_core_n]
        mask = idx[c] < N
        out_full[idx[c][mask]] = o.T[mask]
    return out_full
